# revision 12
# baseline (speedup 1.0000x reference)
"""Trainium2 Bass kernel for nn_DeferredRender (4-level bilinear grid_sample sum).

Bilinear-coefficient mega-entry design
--------------------------------------
Key by the "virtual half-cell" h = floor(u*2048) per axis. For every level L
(texture width W_L = 1024 >> L), the cell x0_L = floor(u*W_L - 0.5) is a pure
function of h:  x0_L = (h - 2^L) >> (L+1)  (exact dyadic argument), and the
fraction decomposes as  fx_L = (X + mx_L) * 2^-(L+1)  with X = u*2048 - h and
mx_L = (h - 2^L) mod 2^(L+1)  -- a function of h alone.

Writing each level's bilinear sample as a + fx*dx + fy*dy + fx*fy*dxy and
substituting, the ENTIRE 4-level sum collapses to

    out[c] = A[c] + X*B[c] + Y*C[c] + (X*Y)*D[c]

with A,B,C,D four 8-channel vectors precomputed per (hy, hx) key on the host
(the mx/my cross terms fold into A,B,C). One 64-byte gather per pixel and six
vector ops per tile replace all per-level weight machinery. The algebra is an
exact reparameterization, valid for either tie-break of h = round(u*2048-0.5),
since X is computed against the same h.

Device kernel (per core, 256 of 2048 rows, H-sharded 8 ways): per [128 x KK]
pixel block, compute (hx, hy, X, Y, XY, idx) on DVE/ACT, fetch one 64B entry
per pixel via SWDGE indirect DMA ([128,1] indices per instruction -- the only
HW-supported form -- rotated across NQ SWDGE queues), then 3 broadcast-muls +
3 adds, store channels-last fp16; host transposes.
"""

import numpy as np

C = 8
FULL_H = 2048
FULL_W = 2048
N_CORES = 8
ROWS = FULL_H // N_CORES  # 256
KK = 128    # pixels per block column chunk
NQ = 1      # SWDGE queues to rotate indirect DMAs across
BUFS = 4    # tile-pool double-buffering depth

G = 2048    # half-cell grid
ENT = 32    # fp16 elems per entry: A(8) B(8) C(8) D(8)

_CACHED = {}


def _build_coeff_table(tex0, tex1, tex2, tex3):
    """[G*G, 32] fp16: per (hy, hx) the A,B,C,D 8-channel coefficients."""
    texs = [np.asarray(t, np.float32) for t in (tex0, tex1, tex2, tex3)]
    h = np.arange(G)
    A = np.zeros((G, G, C), np.float32)
    B = np.zeros((G, G, C), np.float32)
    Cc = np.zeros((G, G, C), np.float32)
    D = np.zeros((G, G, C), np.float32)
    for L, tex in enumerate(texs):
        W = tex.shape[2]  # == tex.shape[1]
        two = 1 << L
        den = 1 << (L + 1)
        s = 1.0 / den
        x0 = (h - two) >> (L + 1)          # [G] in [-1, W-1]
        m = (h - two) - (x0 << (L + 1))    # [G] in [0, den)
        ms = (m * s).astype(np.float32)    # [G]
        t = tex.transpose(1, 2, 0)         # [H, W, C]

        def rows(yi):
            v = (yi >= 0) & (yi < W)
            return t[np.clip(yi, 0, W - 1)] * v[:, None, None]

        r0 = rows(x0)          # [G, W, C] rows y0 (zero OOB)
        r1 = rows(x0 + 1)      # rows y0+1

        def cols(r, xi):
            v = (xi >= 0) & (xi < W)
            return r[:, np.clip(xi, 0, W - 1)] * v[None, :, None]

        c00 = cols(r0, x0)
        c10 = cols(r0, x0 + 1)
        c01 = cols(r1, x0)
        c11 = cols(r1, x0 + 1)
        del r0, r1
        dx = c10 - c00
        dy = c01 - c00
        dxy = c11 - c10 - c01 + c00
        del c10, c01, c11
        a = c00
        del c00
        msx = ms[None, :, None]
        msy = ms[:, None, None]
        A += a + msx * dx + msy * dy + (msy * msx) * dxy
        B += s * (dx + msy * dxy)
        Cc += s * (dy + msx * dxy)
        D += (s * s) * dxy
        del a, dx, dy, dxy
    out = np.empty((G, G, 4, C), np.float16)
    out[:, :, 0] = A
    out[:, :, 1] = B
    out[:, :, 2] = Cc
    out[:, :, 3] = D
    return np.ascontiguousarray(out.reshape(G * G, 4 * C))


def _build_nc(rows, width):
    import concourse.bacc as bacc
    import concourse.bass as bass
    import concourse.mybir as mybir
    import concourse.tile as tile

    f32 = mybir.dt.float32
    f16 = mybir.dt.float16
    i32 = mybir.dt.int32
    Copy = mybir.ActivationFunctionType.Copy
    MUL = mybir.AluOpType.mult
    ADD = mybir.AluOpType.add
    SUB = mybir.AluOpType.subtract

    nc = bacc.Bacc("TRN2", target_bir_lowering=False, debug=False,
                   num_devices=N_CORES, num_swdge_queues=NQ,
                   dynamic_dma_scratch_size=65536)
    u_d = nc.dram_tensor("u", [rows, width], f32, kind="ExternalInput")
    v_d = nc.dram_tensor("v", [rows, width], f32, kind="ExternalInput")
    tbl_d = nc.dram_tensor("tbl", [G * G, ENT], f16, kind="ExternalInput")
    out_d = nc.dram_tensor("out", [rows, width * C], f16,
                           kind="ExternalOutput")

    with tile.TileContext(nc) as tc:
        with tc.tile_pool(name="main", bufs=BUFS) as pool:
            for r0 in range(0, rows, 128):
                for w0 in range(0, width, KK):
                    u_t = pool.tile([128, KK], f32, tag="u")
                    v_t = pool.tile([128, KK], f32, tag="v")
                    nc.sync.dma_start(u_t[:], u_d.ap()[r0:r0 + 128,
                                                       w0:w0 + KK])
                    nc.sync.dma_start(v_t[:], v_d.ap()[r0:r0 + 128,
                                                       w0:w0 + KK])

                    def cell(src, tagp):
                        """h = round(u*2048 - 0.5); X = u*2048 - h."""
                        s2 = pool.tile([128, KK], f32, tag=f"s2{tagp}")
                        nc.scalar.activation(s2[:], src[:], Copy,
                                             bias=-0.5, scale=float(G))
                        hi = pool.tile([128, KK], i32, tag=f"hi{tagp}")
                        nc.vector.tensor_copy(hi[:], s2[:])
                        nc.vector.tensor_scalar_max(hi[:], hi[:], 0)
                        hf = pool.tile([128, KK], f32, tag=f"hf{tagp}")
                        nc.vector.tensor_copy(hf[:], hi[:])
                        X = pool.tile([128, KK], f32, tag=f"X{tagp}")
                        nc.vector.scalar_tensor_tensor(
                            out=X[:], in0=s2[:], scalar=0.5, in1=hf[:],
                            op0=ADD, op1=SUB)
                        return hi, X

                    hxi, X = cell(u_t, "x")
                    hyi, Y = cell(v_t, "y")

                    XY = pool.tile([128, KK], f32, tag="XY")
                    nc.vector.tensor_mul(XY[:], X[:], Y[:])
                    # idx pre-scaled to element units (coef=1 in the DMA)
                    idx = pool.tile([128, KK], i32, tag="idx")
                    nc.vector.scalar_tensor_tensor(
                        out=idx[:], in0=hyi[:], scalar=G, in1=hxi[:],
                        op0=MUL, op1=ADD)
                    nc.vector.tensor_scalar_mul(idx[:], idx[:], ENT)

                    def indirect_q(out_ap, in_ap_full, off_ap, qname):
                        """indirect_dma_start clone with a selectable SWDGE
                        queue (the library hardcodes qPoolDynamic)."""
                        g = nc.gpsimd
                        out_l = g.lower_ap_dma(out_ap, for_indirect_dma=True)
                        in_l = g.lower_ap_dma(in_ap_full,
                                              for_indirect_dma=True)
                        assert len(in_l) == 1 and len(out_l) == 1
                        off_l = g.lower_ap_dma(off_ap)
                        assert len(off_l) == 1
                        in_l.append(off_l[0])
                        coef = 1  # idx is pre-scaled to element units on DVE
                        in_l[0].dynamic_ap_info = mybir.DynamicAccessPatternInfo(
                            c=0,
                            actual_ap=out_ap.ap,
                            indirect_dim_max_index=in_ap_full.shape[0],
                            offset_expr=[
                                mybir.DynamicAccessPatternOffsetExpr(
                                    coef=coef,
                                    aff_expr=mybir.DynamicAccessPatternOffsetExprAffExpr(
                                        kind="IndirectArgId", arg_id=1),
                                )
                            ],
                        )
                        return g.add_instruction(
                            mybir.InstDMACopy(
                                name=g.bass.get_next_instruction_name(),
                                queue=qname,
                                mode="Copy",
                                ins=in_l,
                                outs=out_l,
                                oob_is_err=True,
                                cce_op=mybir.AluOpType.bypass,
                            ))

                    patch = pool.tile([128, KK * ENT], f16, tag="patch")
                    p3 = patch[:].rearrange("p (k e) -> p k e", e=ENT)
                    for k in range(KK):
                        qname = f"qPoolDynamic{(k % NQ) or ''}"
                        indirect_q(p3[:, k, :], tbl_d.ap(),
                                   idx[:, k:k + 1], qname)

                    pv = patch[:].rearrange("p (k j c) -> p k j c", j=4, c=C)
                    m1 = pool.tile([128, KK * C], f16, tag="m1")
                    m2 = pool.tile([128, KK * C], f16, tag="m2")
                    m3 = pool.tile([128, KK * C], f16, tag="m3")
                    m1v = m1[:].rearrange("p (k c) -> p k c", c=C)
                    m2v = m2[:].rearrange("p (k c) -> p k c", c=C)
                    m3v = m3[:].rearrange("p (k c) -> p k c", c=C)
                    Xb = X[:].unsqueeze(2).broadcast_to([128, KK, C])
                    Yb = Y[:].unsqueeze(2).broadcast_to([128, KK, C])
                    XYb = XY[:].unsqueeze(2).broadcast_to([128, KK, C])
                    nc.vector.tensor_mul(m1v, Xb, pv[:, :, 1, :])
                    nc.vector.tensor_mul(m2v, Yb, pv[:, :, 2, :])
                    nc.vector.tensor_mul(m3v, XYb, pv[:, :, 3, :])
                    # S1 = A + M1 ; S2 = M2 + M3 ; OUT = S1 + S2
                    nc.vector.tensor_add(m1v, m1v, pv[:, :, 0, :])
                    nc.vector.tensor_add(m2v, m2v, m3v)
                    ot = pool.tile([128, KK * C], f16, tag="ot")
                    nc.vector.tensor_add(ot[:], m1[:], m2[:])
                    nc.sync.dma_start(
                        out_d.ap()[r0:r0 + 128,
                                   w0 * C:(w0 + KK) * C], ot[:])
    nc.compile()
    return nc


def _get_nc(key, *args):
    if key not in _CACHED:
        _CACHED[key] = _build_nc(*args)
    return _CACHED[key]


def kernel(uv_tensor, iter_nr, tex0, tex1, tex2, tex3):
    from concourse import bass_utils

    bass_utils.upload_artifacts = lambda tmpdir: "local://" + tmpdir

    uv = np.asarray(uv_tensor, dtype=np.float32)
    assert uv.shape == (1, 2, FULL_H, FULL_W), uv.shape
    tbl = _build_coeff_table(tex0, tex1, tex2, tex3)

    nc = _get_nc("full", ROWS, FULL_W)

    in_maps = []
    for i in range(N_CORES):
        r0 = i * ROWS
        in_maps.append({
            "u": np.ascontiguousarray(uv[0, 0, r0:r0 + ROWS, :]),
            "v": np.ascontiguousarray(uv[0, 1, r0:r0 + ROWS, :]),
            "tbl": tbl,
        })

    res = bass_utils.run_bass_kernel_spmd(
        nc, in_maps, core_ids=list(range(N_CORES)))
    globals()["_LAST_RES"] = res
    parts = []
    for i in range(N_CORES):
        o = res.results[i]["out"].reshape(ROWS, FULL_W, C)
        parts.append(np.transpose(o, (2, 0, 1)).astype(np.float32))
    out = np.concatenate(parts, axis=1)[None]
    return out


# revision 13
# speedup vs baseline: 1.0055x; 1.0055x over previous
"""Trainium2 Bass kernel for nn_DeferredRender (4-level bilinear grid_sample sum).

Bilinear-coefficient mega-entry design
--------------------------------------
Key by the "virtual half-cell" h = floor(u*2048) per axis. For every level L
(texture width W_L = 1024 >> L), the cell x0_L = floor(u*W_L - 0.5) is a pure
function of h:  x0_L = (h - 2^L) >> (L+1)  (exact dyadic argument), and the
fraction decomposes as  fx_L = (X + mx_L) * 2^-(L+1)  with X = u*2048 - h and
mx_L = (h - 2^L) mod 2^(L+1)  -- a function of h alone.

Writing each level's bilinear sample as a + fx*dx + fy*dy + fx*fy*dxy and
substituting, the ENTIRE 4-level sum collapses to

    out[c] = A[c] + X*B[c] + Y*C[c] + (X*Y)*D[c]

with A,B,C,D four 8-channel vectors precomputed per (hy, hx) key on the host
(the mx/my cross terms fold into A,B,C). One 64-byte gather per pixel and six
vector ops per tile replace all per-level weight machinery. The algebra is an
exact reparameterization, valid for either tie-break of h = round(u*2048-0.5),
since X is computed against the same h.

Device kernel (per core, 256 of 2048 rows, H-sharded 8 ways): per [128 x KK]
pixel block, compute (hx, hy, X, Y, XY, idx) on DVE/ACT, fetch one 64B entry
per pixel via SWDGE indirect DMA ([128,1] indices per instruction -- the only
HW-supported form -- rotated across NQ SWDGE queues), then 3 broadcast-muls +
3 adds, store channels-last fp16; host transposes.
"""

import numpy as np

C = 8
FULL_H = 2048
FULL_W = 2048
N_CORES = 8
ROWS = FULL_H // N_CORES  # 256
KK = 128    # pixels per block column chunk
NQ = 4      # SWDGE queues to rotate indirect DMAs across
BUFS = 4    # tile-pool double-buffering depth

G = 2048    # half-cell grid
ENT = 32    # fp16 elems per entry: A(8) B(8) C(8) D(8)

_CACHED = {}


def _build_coeff_table(tex0, tex1, tex2, tex3):
    """[G*G, 32] fp16: per (hy, hx) the A,B,C,D 8-channel coefficients."""
    texs = [np.asarray(t, np.float32) for t in (tex0, tex1, tex2, tex3)]
    h = np.arange(G)
    A = np.zeros((G, G, C), np.float32)
    B = np.zeros((G, G, C), np.float32)
    Cc = np.zeros((G, G, C), np.float32)
    D = np.zeros((G, G, C), np.float32)
    for L, tex in enumerate(texs):
        W = tex.shape[2]  # == tex.shape[1]
        two = 1 << L
        den = 1 << (L + 1)
        s = 1.0 / den
        x0 = (h - two) >> (L + 1)          # [G] in [-1, W-1]
        m = (h - two) - (x0 << (L + 1))    # [G] in [0, den)
        ms = (m * s).astype(np.float32)    # [G]
        t = tex.transpose(1, 2, 0)         # [H, W, C]

        def rows(yi):
            v = (yi >= 0) & (yi < W)
            return t[np.clip(yi, 0, W - 1)] * v[:, None, None]

        r0 = rows(x0)          # [G, W, C] rows y0 (zero OOB)
        r1 = rows(x0 + 1)      # rows y0+1

        def cols(r, xi):
            v = (xi >= 0) & (xi < W)
            return r[:, np.clip(xi, 0, W - 1)] * v[None, :, None]

        c00 = cols(r0, x0)
        c10 = cols(r0, x0 + 1)
        c01 = cols(r1, x0)
        c11 = cols(r1, x0 + 1)
        del r0, r1
        dx = c10 - c00
        dy = c01 - c00
        dxy = c11 - c10 - c01 + c00
        del c10, c01, c11
        a = c00
        del c00
        msx = ms[None, :, None]
        msy = ms[:, None, None]
        A += a + msx * dx + msy * dy + (msy * msx) * dxy
        B += s * (dx + msy * dxy)
        Cc += s * (dy + msx * dxy)
        D += (s * s) * dxy
        del a, dx, dy, dxy
    out = np.empty((G, G, 4, C), np.float16)
    out[:, :, 0] = A
    out[:, :, 1] = B
    out[:, :, 2] = Cc
    out[:, :, 3] = D
    return np.ascontiguousarray(out.reshape(G * G, 4 * C))


def _build_nc(rows, width):
    import concourse.bacc as bacc
    import concourse.bass as bass
    import concourse.mybir as mybir
    import concourse.tile as tile

    f32 = mybir.dt.float32
    f16 = mybir.dt.float16
    i32 = mybir.dt.int32
    Copy = mybir.ActivationFunctionType.Copy
    MUL = mybir.AluOpType.mult
    ADD = mybir.AluOpType.add
    SUB = mybir.AluOpType.subtract

    nc = bacc.Bacc("TRN2", target_bir_lowering=False, debug=False,
                   num_devices=N_CORES, num_swdge_queues=NQ,
                   dynamic_dma_scratch_size=65536)
    u_d = nc.dram_tensor("u", [rows, width], f32, kind="ExternalInput")
    v_d = nc.dram_tensor("v", [rows, width], f32, kind="ExternalInput")
    tbl_d = nc.dram_tensor("tbl", [G * G, ENT], f16, kind="ExternalInput")
    out_d = nc.dram_tensor("out", [rows, width * C], f16,
                           kind="ExternalOutput")

    with tile.TileContext(nc) as tc:
        with tc.tile_pool(name="main", bufs=BUFS) as pool:
            for r0 in range(0, rows, 128):
                for w0 in range(0, width, KK):
                    u_t = pool.tile([128, KK], f32, tag="u")
                    v_t = pool.tile([128, KK], f32, tag="v")
                    nc.sync.dma_start(u_t[:], u_d.ap()[r0:r0 + 128,
                                                       w0:w0 + KK])
                    nc.sync.dma_start(v_t[:], v_d.ap()[r0:r0 + 128,
                                                       w0:w0 + KK])

                    def cell(src, tagp):
                        """h = round(u*2048 - 0.5); X = u*2048 - h."""
                        s2 = pool.tile([128, KK], f32, tag=f"s2{tagp}")
                        nc.scalar.activation(s2[:], src[:], Copy,
                                             bias=-0.5, scale=float(G))
                        hi = pool.tile([128, KK], i32, tag=f"hi{tagp}")
                        nc.vector.tensor_copy(hi[:], s2[:])
                        nc.vector.tensor_scalar_max(hi[:], hi[:], 0)
                        hf = pool.tile([128, KK], f32, tag=f"hf{tagp}")
                        nc.vector.tensor_copy(hf[:], hi[:])
                        X = pool.tile([128, KK], f32, tag=f"X{tagp}")
                        nc.vector.scalar_tensor_tensor(
                            out=X[:], in0=s2[:], scalar=0.5, in1=hf[:],
                            op0=ADD, op1=SUB)
                        return hi, X

                    hxi, X = cell(u_t, "x")
                    hyi, Y = cell(v_t, "y")

                    XY = pool.tile([128, KK], f32, tag="XY")
                    nc.vector.tensor_mul(XY[:], X[:], Y[:])
                    # idx pre-scaled to element units (coef=1 in the DMA)
                    idx = pool.tile([128, KK], i32, tag="idx")
                    nc.vector.scalar_tensor_tensor(
                        out=idx[:], in0=hyi[:], scalar=G, in1=hxi[:],
                        op0=MUL, op1=ADD)
                    nc.vector.tensor_scalar_mul(idx[:], idx[:], ENT)

                    def indirect_q(out_ap, in_ap_full, off_ap, qname):
                        """indirect_dma_start clone with a selectable SWDGE
                        queue (the library hardcodes qPoolDynamic)."""
                        g = nc.gpsimd
                        out_l = g.lower_ap_dma(out_ap, for_indirect_dma=True)
                        in_l = g.lower_ap_dma(in_ap_full,
                                              for_indirect_dma=True)
                        assert len(in_l) == 1 and len(out_l) == 1
                        off_l = g.lower_ap_dma(off_ap)
                        assert len(off_l) == 1
                        in_l.append(off_l[0])
                        coef = 1  # idx is pre-scaled to element units on DVE
                        in_l[0].dynamic_ap_info = mybir.DynamicAccessPatternInfo(
                            c=0,
                            actual_ap=out_ap.ap,
                            indirect_dim_max_index=in_ap_full.shape[0],
                            offset_expr=[
                                mybir.DynamicAccessPatternOffsetExpr(
                                    coef=coef,
                                    aff_expr=mybir.DynamicAccessPatternOffsetExprAffExpr(
                                        kind="IndirectArgId", arg_id=1),
                                )
                            ],
                        )
                        return g.add_instruction(
                            mybir.InstDMACopy(
                                name=g.bass.get_next_instruction_name(),
                                queue=qname,
                                mode="Copy",
                                ins=in_l,
                                outs=out_l,
                                oob_is_err=True,
                                cce_op=mybir.AluOpType.bypass,
                            ))

                    patch = pool.tile([128, KK * ENT], f16, tag="patch")
                    p3 = patch[:].rearrange("p (k e) -> p k e", e=ENT)
                    for k in range(KK):
                        qname = f"qPoolDynamic{(k % NQ) or ''}"
                        indirect_q(p3[:, k, :], tbl_d.ap(),
                                   idx[:, k:k + 1], qname)

                    pv = patch[:].rearrange("p (k j c) -> p k j c", j=4, c=C)
                    m1 = pool.tile([128, KK * C], f16, tag="m1")
                    m2 = pool.tile([128, KK * C], f16, tag="m2")
                    m3 = pool.tile([128, KK * C], f16, tag="m3")
                    m1v = m1[:].rearrange("p (k c) -> p k c", c=C)
                    m2v = m2[:].rearrange("p (k c) -> p k c", c=C)
                    m3v = m3[:].rearrange("p (k c) -> p k c", c=C)
                    Xb = X[:].unsqueeze(2).broadcast_to([128, KK, C])
                    Yb = Y[:].unsqueeze(2).broadcast_to([128, KK, C])
                    XYb = XY[:].unsqueeze(2).broadcast_to([128, KK, C])
                    nc.vector.tensor_mul(m1v, Xb, pv[:, :, 1, :])
                    nc.vector.tensor_mul(m2v, Yb, pv[:, :, 2, :])
                    nc.vector.tensor_mul(m3v, XYb, pv[:, :, 3, :])
                    # S1 = A + M1 ; S2 = M2 + M3 ; OUT = S1 + S2
                    nc.vector.tensor_add(m1v, m1v, pv[:, :, 0, :])
                    nc.vector.tensor_add(m2v, m2v, m3v)
                    ot = pool.tile([128, KK * C], f16, tag="ot")
                    nc.vector.tensor_add(ot[:], m1[:], m2[:])
                    nc.sync.dma_start(
                        out_d.ap()[r0:r0 + 128,
                                   w0 * C:(w0 + KK) * C], ot[:])
    nc.compile()
    return nc


def _get_nc(key, *args):
    if key not in _CACHED:
        _CACHED[key] = _build_nc(*args)
    return _CACHED[key]


def kernel(uv_tensor, iter_nr, tex0, tex1, tex2, tex3):
    from concourse import bass_utils

    bass_utils.upload_artifacts = lambda tmpdir: "local://" + tmpdir

    uv = np.asarray(uv_tensor, dtype=np.float32)
    assert uv.shape == (1, 2, FULL_H, FULL_W), uv.shape
    tbl = _build_coeff_table(tex0, tex1, tex2, tex3)

    nc = _get_nc("full", ROWS, FULL_W)

    in_maps = []
    for i in range(N_CORES):
        r0 = i * ROWS
        in_maps.append({
            "u": np.ascontiguousarray(uv[0, 0, r0:r0 + ROWS, :]),
            "v": np.ascontiguousarray(uv[0, 1, r0:r0 + ROWS, :]),
            "tbl": tbl,
        })

    res = bass_utils.run_bass_kernel_spmd(
        nc, in_maps, core_ids=list(range(N_CORES)))
    globals()["_LAST_RES"] = res
    parts = []
    for i in range(N_CORES):
        o = res.results[i]["out"].reshape(ROWS, FULL_W, C)
        parts.append(np.transpose(o, (2, 0, 1)).astype(np.float32))
    out = np.concatenate(parts, axis=1)[None]
    return out
